# revision 26
# baseline (speedup 1.0000x reference)
"""Trainium2 Bass kernel for nn_GameTensor_27195732918735.

Computes out[i,j,b] = Hessian_z V_i(z_all[j,b]) for i != j, zeros on the
diagonal, where V_i(z) = W2[i] @ tanh(W1[i] @ z + b1[i]) + b2[i].

Analytic form used on-device:
    u = W1 z + b1;  th = tanh(u);  s_k = -2 W2_k th_k (1 - th_k^2)
    H = W1^T diag(s) W1  =  sum_k s_k w1_k w1_k^T

H is symmetric, so the device only computes one entry per unordered pair
(a, c).  Pairs are packed by circular diagonal: slot d in 0..64 holds
T[k, d, a] = W1[k, a] * W1[k, (a + d) % 128], built on DVE from a doubled
copy of W1 (plus a one-shifted copy for odd d, keeping every operand
stride-1 and 4B-aligned so the bf16 2x_1P perf mode engages).  Per task the
Hessians for 128 batches are then H[b, col] = sum_k S[k, b] T[k, col]
(bf16 matmuls, fp32 PSUM), staged to SBUF as bf16 and DMAd out.  The host
mirrors the packed pairs into the full [B, D, D] blocks with a gather LUT
and writes the diagonal zero blocks (both pure data movement).

Engine notes baked into the structure (measured on TRN2):
  - GPSIMD is unused: it cannot read PSUM, and any concurrent GPSIMD
    execution slows DVE/Scalar ops several-fold.
  - Only Scalar and Vector can read PSUM; the 24 group copies are split
    between them (CP_SCHED), Scalar-heavy early while DVE builds TT.
  - Input DMAs are merged into 3 transfers to cut Sync issue latency.
  - PE warmup matmuls ramp the p-state before real work arrives; a dummy
    tanh preloads the activation table.

Per-core plan (8 cores, SPMD): core c owns agent i = c//2 and three
(j, batch-half) tasks (12 nonzero (i,j) cells x 2 halves = 24 / 8 = 3).
"""

import numpy as np
import ml_dtypes

import concourse.bass as bass
import concourse.mybir as mybir
import concourse.tile as tile
from concourse import bacc
from concourse.bass_utils import run_bass_kernel_spmd

# ---- custom fused DVE op: s = C0 * (x - x^3) = C0 * x * (1 - x^2) ----------
import concourse.dve_ops as _dve_ops
from concourse.dve_ops import DveOp as _DveOp, DveOpSpec as _DveOpSpec, OPS as _OPS
from concourse.dve_spec import Spec as _Spec, Src0 as _Src0, C0 as _C0
from concourse.dve_spec import lower as _dve_lower


def _register_tanh_hess_op():
    name = "TANH_HESS_S_ANT"
    for op in _OPS:
        if op.name == name:
            return op
    spec = _Spec(
        body=(_Src0 - _Src0 * _Src0 * _Src0) * _C0,
        reference=lambda in0, s0: (in0 - in0**3) * s0,
    )
    _dve_ops._SUB_OPCODE_FOR_NAME[name] = _dve_ops._CUSTOM_DVE_ROW_BASE + len(_OPS)
    shas = {}
    for ver in ("v3", "v4"):
        s = _DveOpSpec(
            name=name,
            opcode=_dve_ops._SUB_OPCODE_FOR_NAME[name],
            uops=_dve_lower(spec, ver=ver),
            rd1_en=False,
        )
        shas[ver] = s.sha(ver)
    op = _DveOp(name, spec, subdim=False, uops_sha=shas)
    _OPS.append(op)
    _dve_ops.CUSTOM_DVE_SPECS[name] = spec
    return op


_TANH_HESS_S = _register_tanh_hess_op()

N, B, D = 4, 256, 128
H2 = 2 * D  # 256 hidden
NCORES = 8
NTASK = 3  # (j, half) tasks per core
HALF = B // 2  # 128 batches per task

# Packed-pair layout: 65 diagonal slots of 128 columns.
# Column order: [E0 O0 E1 O1 E2 O2 E3 O3 | TAIL] where E-chunk e holds even
# d = 16e..16e+14 (8 slots), O-chunk o holds odd d = 16o+1..16o+15 (8 slots),
# TAIL is the single d=64 slot. Total 8*1024 + 128 = 8320 columns.
NSLOT = 65
COLS = NSLOT * 128  # 8320
NCHUNK = 4  # E/O chunk pairs
TAILCOL = 8192

MM_MODE = "bf16"  # kept for test-harness compat; bf16 is the only mode

_F32 = mybir.dt.float32
_BF16 = mybir.dt.bfloat16

_AP = None  # bass_rust.AP class, resolved lazily


def _win_ap(tile_ap, base_off, nd, dstep):
    """Overlapping sliding-window AP: [128p][nd windows, stride dstep][128, 1].

    tile_ap must be a [128, R] view of an SBUF tile. Window w reads elements
    base_off + w*dstep + 0..127 of the view.
    """
    global _AP
    if _AP is None:
        _AP = type(tile_ap)
    pdim = [int(v) for v in list(tile_ap.ap)[0]]
    return _AP(
        tensor=tile_ap.tensor,
        offset=int(tile_ap.offset) + base_off,
        ap=[pdim, [dstep, nd], [1, 128]],
    )


# 24 big PSUM->SBUF copies: A = Scalar (closer to PSUM), D = Vector.
# Scalar-heavy early (DVE still building TT chunks), balanced later; the
# final stage's pair is A,D so the two copies run in parallel at the end.
CP_SCHED = "AADAAD" "AADAAD" "ADADAD" "ADADAD"


def _emit(tc, nc, w1x, w1zt, cvec, out):
    Tanh = mybir.ActivationFunctionType.Tanh
    Ident = mybir.ActivationFunctionType.Identity
    mult = mybir.AluOpType.mult
    add = mybir.AluOpType.add

    with (
        tc.tile_pool(name="consts", bufs=1) as consts,
        tc.tile_pool(name="tpool", bufs=1) as tpool,
        tc.tile_pool(name="small", bufs=2) as small,
        tc.tile_pool(name="warm", bufs=1) as warm,
        tc.tile_pool(name="stage", bufs=6) as stage_pool,
        tc.tile_pool(name="tstage", bufs=1) as tstage_pool,
        tc.tile_pool(name="upsum", bufs=2, space="PSUM") as upsum,
        tc.tile_pool(name="psum", bufs=3, space="PSUM") as psum,
    ):
        # ---- merged input DMAs ----------------------------------------------
        w1zt_sb = consts.tile([128, 640], _BF16)  # [d, w1t(256) | zt(3x128)]
        nc.scalar.dma_start(w1zt_sb, w1zt)
        cv_sb = consts.tile([128, 6], _F32)  # [b1(2) | w2s(2) | w2n(2)]
        nc.scalar.dma_start(cv_sb, cvec)
        w1x_sb = consts.tile([128, 2, 2, 256], _BF16)  # [p, dbl/shf, kc, a]
        nc.sync.dma_start(w1x_sb, w1x)

        # ---- warmups: PE p-state ramp + Tanh act-table preload --------------
        wz = warm.tile([128, 512], _BF16)
        nc.vector.memset(wz, 0)
        for _ in range(3):
            wps = psum.tile([128, 1024], _F32, tag="ps")
            nc.tensor.matmul(
                wps[:, 0:512], lhsT=wz[:, 0:128], rhs=wz, start=True, stop=True
            )
        wt = warm.tile([128, 8], _F32)
        nc.scalar.memzero(wt)
        wth = warm.tile([128, 8], _F32)
        nc.scalar.activation(wth, wt, Tanh, bias=0.0)

        TT = tpool.tile([128, 2, COLS], _BF16)

        def tt_chunk(ch, par, kc):
            col0 = ch * 2048 + par * 1024
            dst = TT[:, kc, col0 : col0 + 1024].rearrange("p (w x) -> p w x", w=8)
            in0 = w1x_sb[:, 0, kc, None, 0:128].to_broadcast((128, 8, 128))
            in1 = _win_ap(w1x_sb[:, par, kc, :], 16 * ch, 8, 2)
            nc.vector.tensor_tensor(dst, in0, in1, mult)

        # ---- TT: tail slot (d=64) + first chunk pair on DVE -----------------
        for kc in range(2):
            nc.vector.tensor_tensor(
                TT[:, kc, TAILCOL : TAILCOL + 128],
                w1x_sb[:, 0, kc, 0:128],
                w1x_sb[:, 0, kc, 64:192],
                mult,
            )
        for kc in range(2):
            tt_chunk(0, 0, kc)  # E0
        for kc in range(2):
            tt_chunk(0, 1, kc)  # O0

        # ---- S batched over tasks: s = th * (w2s*(1 - th^2)) ----------------
        # th, sq = th^2, g = w2s - w2s*sq on Scalar (per-partition scale and
        # bias APs); one DVE multiply s = th*g per kc.
        s_sb = consts.tile([128, 2, NTASK * 128], _BF16)
        for kc in range(2):
            ups = upsum.tile([128, NTASK * 128], _F32, tag="ups")
            nc.tensor.matmul(
                ups,
                lhsT=w1zt_sb[:, kc * 128 : (kc + 1) * 128],
                rhs=w1zt_sb[:, 256:640],
                start=True,
                stop=True,
            )
            th = small.tile([128, NTASK * 128], _F32, tag="th")
            nc.scalar.activation(th, ups, Tanh, bias=cv_sb[:, kc : kc + 1])
            with tc.high_priority():
                # fused: s = w2s * (th - th^3) in one DVE pass
                nc.vector._custom_dve(
                    _TANH_HESS_S,
                    out=s_sb[:, kc, :],
                    in0=th,
                    s0=cv_sb[:, 2 + kc : 3 + kc],
                )

        # extra warmups keep the PE p-state hot until real groups arrive
        for _ in range(6):
            wps = psum.tile([128, 1024], _F32, tag="ps")
            nc.tensor.matmul(
                wps[:, 0:512], lhsT=wz[:, 0:128], rhs=wz, start=True, stop=True
            )

        # The first 3 psum groups start their kc0 matmuls as soon as s[kc0]
        # is ready (kc1 accumulates later), filling the PE while the kc1
        # S-chain finishes.
        early = []  # (ps, t, g) with kc0 already accumulated
        for t, g in ((0, 0), (0, 1), (1, 0)):
            ps = psum.tile([128, 1024], _F32, tag="ps")
            for nn in range(2):
                c0 = g * 1024 + nn * 512
                nc.tensor.matmul(
                    ps[:, nn * 512 : (nn + 1) * 512],
                    lhsT=s_sb[:, 0, t * 128 : (t + 1) * 128],
                    rhs=TT[:, 0, c0 : c0 + 512],
                    start=True,
                    stop=False,
                )
            early.append(((t, g), ps))
        early = dict(early)

        # ---- tail groups (cols 8192..8319) for all tasks: done early --------
        tstage = tstage_pool.tile([128, NTASK, 128], _BF16)
        for t in range(NTASK):
            ps = upsum.tile([128, 128], _F32, tag="ups")
            nc.tensor.matmul(
                ps,
                lhsT=s_sb[:, 0, t * 128 : (t + 1) * 128],
                rhs=TT[:, 0, TAILCOL:],
                start=True,
                stop=False,
            )
            nc.tensor.matmul(
                ps,
                lhsT=s_sb[:, 1, t * 128 : (t + 1) * 128],
                rhs=TT[:, 1, TAILCOL:],
                start=False,
                stop=True,
            )
            nc.scalar.copy(tstage[:, t, :], ps)
        nc.sync.dma_start(out[:, :, TAILCOL:].rearrange("t b c -> b t c"), tstage)

        # ---- main loop: 4 gp x 3 tasks x 2 groups; prefetch next TT chunks --
        cp_i = 0
        for gp in range(4):
            if gp + 1 < NCHUNK:
                for par in range(2):
                    for kc in range(2):
                        tt_chunk(gp + 1, par, kc)
            for t in range(NTASK):
                last = gp == 3 and t == NTASK - 1
                stg = stage_pool.tile([128, 2048], _BF16)
                for half in range(2):
                    g = gp * 2 + half
                    col0 = g * 1024
                    pre = early.get((t, g))
                    ps = pre if pre is not None else psum.tile(
                        [128, 1024], _F32, tag="ps"
                    )
                    for nn in range(2):
                        sl = slice(nn * 512, (nn + 1) * 512)
                        c0 = col0 + nn * 512
                        if pre is None:
                            nc.tensor.matmul(
                                ps[:, sl],
                                lhsT=s_sb[:, 0, t * 128 : (t + 1) * 128],
                                rhs=TT[:, 0, c0 : c0 + 512],
                                start=True,
                                stop=False,
                            )
                        nc.tensor.matmul(
                            ps[:, sl],
                            lhsT=s_sb[:, 1, t * 128 : (t + 1) * 128],
                            rhs=TT[:, 1, c0 : c0 + 512],
                            start=False,
                            stop=True,
                        )
                    dst = stg[:, half * 1024 : (half + 1) * 1024]
                    if last:
                        # split the final copies 512-wide across both engines
                        # so the drain runs in parallel
                        nc.scalar.copy(dst[:, 0:512], ps[:, 0:512])
                        nc.vector.tensor_copy(
                            out=dst[:, 512:1024], in_=ps[:, 512:1024]
                        )
                        nc.sync.dma_start(
                            out[t, :, gp * 2048 + half * 1024 :
                                gp * 2048 + (half + 1) * 1024],
                            dst,
                        )
                    else:
                        if CP_SCHED[cp_i] == "A":
                            nc.scalar.copy(dst, ps)
                        else:
                            nc.vector.tensor_copy(out=dst, in_=ps)
                    cp_i += 1
                if not last:
                    nc.sync.dma_start(out[t, :, gp * 2048 : (gp + 1) * 2048], stg)


_NC_CACHE = {}


def _core_tasks(c):
    i = c // 2
    js = [j for j in range(N) if j != i]
    halves = [(j, h) for j in js for h in (0, 1)]
    return i, (halves[0:3] if c % 2 == 0 else halves[3:6])


def _build():
    key = "v4"
    if key in _NC_CACHE:
        return _NC_CACHE[key]
    nc = bacc.Bacc("TRN2", target_bir_lowering=False, debug=False, num_devices=NCORES)
    w1x = nc.dram_tensor("w1x", [128, 2, 2, 256], _BF16, kind="ExternalInput").ap()
    w1zt = nc.dram_tensor("w1zt", [128, 640], _BF16, kind="ExternalInput").ap()
    cvec = nc.dram_tensor("cvec", [128, 6], _F32, kind="ExternalInput").ap()
    out = nc.dram_tensor("out", [NTASK, HALF, COLS], _BF16, kind="ExternalOutput").ap()
    with tile.TileContext(nc) as tc:
        _emit(tc, nc, w1x, w1zt, cvec, out)
    nc.compile()
    _NC_CACHE[key] = nc
    return nc


def _slot_col(d):
    """Column of diagonal-slot d in the packed layout."""
    if d == 64:
        return TAILCOL
    if d % 2 == 0:
        de = d // 2
        return (de // 8) * 2048 + (de % 8) * 128
    do = (d - 1) // 2
    return (do // 8) * 2048 + 1024 + (do % 8) * 128


_LUT = None


def _lut():
    global _LUT
    if _LUT is None:
        a = np.arange(128)[:, None]
        c = np.arange(128)[None, :]
        g = (c - a) % 128
        d = np.where(g <= 64, g, 128 - g)
        base_a = np.where(g <= 64, np.broadcast_to(a, (128, 128)), c)
        slot = np.vectorize(_slot_col)(d)
        _LUT = (slot + base_a).astype(np.int32)
    return _LUT


# Options for test harness introspection (set by test.py, unused in grading).
_RUN_KWARGS = {}
_LAST_RESULT = None


def kernel(z_all, W1, b1, W2, b2):
    global _LAST_RESULT
    z_all = np.asarray(z_all, dtype=np.float32)
    W1 = np.asarray(W1, dtype=np.float32)
    b1 = np.asarray(b1, dtype=np.float32)
    W2 = np.asarray(W2, dtype=np.float32)

    nc = _build()
    bf = ml_dtypes.bfloat16

    in_maps = []
    metas = []
    for c in range(NCORES):
        i, tasks = _core_tasks(c)
        metas.append((i, tasks))
        w1i = W1[i].astype(bf)  # [256, 128]
        w1ck = w1i.reshape(2, 128, 128).transpose(1, 0, 2)  # [k%128, kc, a]
        w1dbl = np.concatenate([w1ck, w1ck], axis=2)  # [128, 2, 256]
        w1shf = np.concatenate(
            [w1ck[:, :, 1:], w1ck[:, :, :1], w1ck[:, :, 1:], w1ck[:, :, :1]], axis=2
        )  # shifted: w1o[p, kc, j] = w1[p, kc, (j+1)%128]
        w1x = np.stack([w1dbl, w1shf], axis=1)  # [128, 2, 2, 256]
        ztm = np.concatenate(
            [z_all[j, h * HALF : (h + 1) * HALF, :].T for (j, h) in tasks], axis=1
        ).astype(bf)  # [128 d, 384]
        w1zt = np.concatenate([w1i.T, ztm], axis=1)  # [128, 640]
        cvec = np.stack(
            [
                b1[i][:128], b1[i][128:],
                -2.0 * W2[i, 0, :128], -2.0 * W2[i, 0, 128:],
                2.0 * W2[i, 0, :128], 2.0 * W2[i, 0, 128:],
            ],
            axis=1,
        ).astype(np.float32)  # [128, 6]
        in_maps.append(
            {
                "w1x": np.ascontiguousarray(w1x),
                "w1zt": np.ascontiguousarray(w1zt),
                "cvec": np.ascontiguousarray(cvec),
            }
        )

    res = run_bass_kernel_spmd(nc, in_maps, list(range(NCORES)), **_RUN_KWARGS)
    _LAST_RESULT = res

    lut = _lut()
    full = np.zeros((N, N, B, D, D), dtype=np.float32)
    for c in range(NCORES):
        i, tasks = metas[c]
        o = np.asarray(res.results[c]["out"]).astype(np.float32)  # [NTASK, HALF, COLS]
        for t, (j, h) in enumerate(tasks):
            full[i, j, h * HALF : (h + 1) * HALF] = o[t][:, lut]
    return full


# revision 27
# speedup vs baseline: 1.1664x; 1.1664x over previous
"""Trainium2 Bass kernel for nn_GameTensor_27195732918735.

Computes out[i,j,b] = Hessian_z V_i(z_all[j,b]) for i != j, zeros on the
diagonal, where V_i(z) = W2[i] @ tanh(W1[i] @ z + b1[i]) + b2[i].

Analytic form used on-device:
    u = W1 z + b1;  th = tanh(u);  s_k = -2 W2_k th_k (1 - th_k^2)
    H = W1^T diag(s) W1  =  sum_k s_k w1_k w1_k^T

H is symmetric, so the device only computes one entry per unordered pair
(a, c).  Pairs are packed by circular diagonal: slot d in 0..64 holds
T[k, d, a] = W1[k, a] * W1[k, (a + d) % 128], built on DVE from a doubled
copy of W1 (plus a one-shifted copy for odd d, keeping every operand
stride-1 and 4B-aligned so the bf16 2x_1P perf mode engages).  Per task the
Hessians for 128 batches are then H[b, col] = sum_k S[k, b] T[k, col]
(bf16 matmuls, fp32 PSUM), staged to SBUF as bf16 and DMAd out.  The host
mirrors the packed pairs into the full [B, D, D] blocks with a gather LUT
and writes the diagonal zero blocks (both pure data movement).

Engine notes baked into the structure (measured on TRN2):
  - GPSIMD is unused: it cannot read PSUM, and any concurrent GPSIMD
    execution slows DVE/Scalar ops several-fold.
  - Only Scalar and Vector can read PSUM; the 24 group copies are split
    between them (CP_SCHED), Scalar-heavy early while DVE builds TT.
  - Input DMAs are merged into 3 transfers to cut Sync issue latency.
  - PE warmup matmuls ramp the p-state before real work arrives; a dummy
    tanh preloads the activation table.

Per-core plan (8 cores, SPMD): core c owns agent i = c//2 and three
(j, batch-half) tasks (12 nonzero (i,j) cells x 2 halves = 24 / 8 = 3).
"""

import numpy as np
import ml_dtypes

import concourse.bass as bass
import concourse.mybir as mybir
import concourse.tile as tile
from concourse import bacc
from concourse.bass_utils import run_bass_kernel_spmd

# ---- custom fused DVE op: s = C0 * (x - x^3) = C0 * x * (1 - x^2) ----------
import concourse.dve_ops as _dve_ops
from concourse.dve_ops import DveOp as _DveOp, DveOpSpec as _DveOpSpec, OPS as _OPS
from concourse.dve_spec import Spec as _Spec, Src0 as _Src0, C0 as _C0
from concourse.dve_spec import lower as _dve_lower


def _register_tanh_hess_op():
    name = "TANH_HESS_S_ANT"
    for op in _OPS:
        if op.name == name:
            return op
    spec = _Spec(
        body=(_Src0 - _Src0 * _Src0 * _Src0) * _C0,
        reference=lambda in0, s0: (in0 - in0**3) * s0,
    )
    _dve_ops._SUB_OPCODE_FOR_NAME[name] = _dve_ops._CUSTOM_DVE_ROW_BASE + len(_OPS)
    shas = {}
    for ver in ("v3", "v4"):
        s = _DveOpSpec(
            name=name,
            opcode=_dve_ops._SUB_OPCODE_FOR_NAME[name],
            uops=_dve_lower(spec, ver=ver),
            rd1_en=False,
        )
        shas[ver] = s.sha(ver)
    op = _DveOp(name, spec, subdim=False, uops_sha=shas)
    _OPS.append(op)
    _dve_ops.CUSTOM_DVE_SPECS[name] = spec
    return op


_TANH_HESS_S = _register_tanh_hess_op()

N, B, D = 4, 256, 128
H2 = 2 * D  # 256 hidden
NCORES = 8
NTASK = 3  # (j, half) tasks per core
HALF = B // 2  # 128 batches per task

# Packed-pair layout: 65 diagonal slots of 128 columns.
# Column order: [E0 O0 E1 O1 E2 O2 E3 O3 | TAIL] where E-chunk e holds even
# d = 16e..16e+14 (8 slots), O-chunk o holds odd d = 16o+1..16o+15 (8 slots),
# TAIL is the single d=64 slot. Total 8*1024 + 128 = 8320 columns.
NSLOT = 65
COLS = NSLOT * 128  # 8320
NCHUNK = 4  # E/O chunk pairs
TAILCOL = 8192

MM_MODE = "bf16"  # kept for test-harness compat; bf16 is the only mode

_F32 = mybir.dt.float32
_BF16 = mybir.dt.bfloat16

_AP = None  # bass_rust.AP class, resolved lazily


def _win_ap(tile_ap, base_off, nd, dstep):
    """Overlapping sliding-window AP: [128p][nd windows, stride dstep][128, 1].

    tile_ap must be a [128, R] view of an SBUF tile. Window w reads elements
    base_off + w*dstep + 0..127 of the view.
    """
    global _AP
    if _AP is None:
        _AP = type(tile_ap)
    pdim = [int(v) for v in list(tile_ap.ap)[0]]
    return _AP(
        tensor=tile_ap.tensor,
        offset=int(tile_ap.offset) + base_off,
        ap=[pdim, [dstep, nd], [1, 128]],
    )


# 24 big PSUM->SBUF copies: A = Scalar (closer to PSUM), D = Vector.
# Scalar-heavy early (DVE still building TT chunks), balanced later; the
# final stage's pair is A,D so the two copies run in parallel at the end.
CP_SCHED = "AADAAD" "AADAAD" "ADADAD" "ADADAD"


def _emit(tc, nc, w1x, w1zt, cvec, out):
    Tanh = mybir.ActivationFunctionType.Tanh
    Ident = mybir.ActivationFunctionType.Identity
    mult = mybir.AluOpType.mult
    add = mybir.AluOpType.add

    with (
        tc.tile_pool(name="consts", bufs=1) as consts,
        tc.tile_pool(name="tpool", bufs=1) as tpool,
        tc.tile_pool(name="small", bufs=2) as small,
        tc.tile_pool(name="warm", bufs=1) as warm,
        tc.tile_pool(name="stage", bufs=6) as stage_pool,
        tc.tile_pool(name="tstage", bufs=1) as tstage_pool,
        tc.tile_pool(name="upsum", bufs=2, space="PSUM") as upsum,
        tc.tile_pool(name="psum", bufs=3, space="PSUM") as psum,
    ):
        # ---- merged input DMAs ----------------------------------------------
        w1zt_sb = consts.tile([128, 640], _BF16)  # [d, w1t(256) | zt(3x128)]
        nc.scalar.dma_start(w1zt_sb, w1zt)
        cv_sb = consts.tile([128, 6], _F32)  # [b1(2) | w2s(2) | w2n(2)]
        nc.scalar.dma_start(cv_sb, cvec)
        w1x_sb = consts.tile([128, 2, 2, 256], _BF16)  # [p, dbl/shf, kc, a]
        nc.sync.dma_start(w1x_sb, w1x)

        # ---- warmups: PE p-state ramp + Tanh act-table preload --------------
        wz = warm.tile([128, 512], _BF16)
        nc.vector.memset(wz, 0)
        for _ in range(4):
            wps = psum.tile([128, 1024], _F32, tag="ps")
            nc.tensor.matmul(
                wps[:, 0:512], lhsT=wz[:, 0:128], rhs=wz, start=True, stop=True
            )
        wt = warm.tile([128, 8], _F32)
        nc.scalar.memzero(wt)
        wth = warm.tile([128, 8], _F32)
        nc.scalar.activation(wth, wt, Tanh, bias=0.0)

        TT = tpool.tile([128, 2, COLS], _BF16)

        def tt_chunk(ch, par, kc):
            col0 = ch * 2048 + par * 1024
            dst = TT[:, kc, col0 : col0 + 1024].rearrange("p (w x) -> p w x", w=8)
            in0 = w1x_sb[:, 0, kc, None, 0:128].to_broadcast((128, 8, 128))
            in1 = _win_ap(w1x_sb[:, par, kc, :], 16 * ch, 8, 2)
            nc.vector.tensor_tensor(dst, in0, in1, mult)

        # ---- TT: tail slot (d=64) + first chunk pair on DVE -----------------
        for kc in range(2):
            nc.vector.tensor_tensor(
                TT[:, kc, TAILCOL : TAILCOL + 128],
                w1x_sb[:, 0, kc, 0:128],
                w1x_sb[:, 0, kc, 64:192],
                mult,
            )
        for kc in range(2):
            tt_chunk(0, 0, kc)  # E0
        for kc in range(2):
            tt_chunk(0, 1, kc)  # O0

        # ---- S batched over tasks: s = th * (w2s*(1 - th^2)) ----------------
        # th, sq = th^2, g = w2s - w2s*sq on Scalar (per-partition scale and
        # bias APs); one DVE multiply s = th*g per kc.
        s_sb = consts.tile([128, 2, NTASK * 128], _BF16)
        for kc in range(2):
            ups = upsum.tile([128, NTASK * 128], _F32, tag="ups")
            nc.tensor.matmul(
                ups,
                lhsT=w1zt_sb[:, kc * 128 : (kc + 1) * 128],
                rhs=w1zt_sb[:, 256:640],
                start=True,
                stop=True,
            )
            th = small.tile([128, NTASK * 128], _F32, tag="th")
            nc.scalar.activation(th, ups, Tanh, bias=cv_sb[:, kc : kc + 1])
            with tc.high_priority():
                # fused: s = w2s * (th - th^3) in one DVE pass
                nc.vector._custom_dve(
                    _TANH_HESS_S,
                    out=s_sb[:, kc, :],
                    in0=th,
                    s0=cv_sb[:, 2 + kc : 3 + kc],
                )

        # extra warmups keep the PE p-state hot until real groups arrive
        for _ in range(8):
            wps = psum.tile([128, 1024], _F32, tag="ps")
            nc.tensor.matmul(
                wps[:, 0:512], lhsT=wz[:, 0:128], rhs=wz, start=True, stop=True
            )

        # The first 3 psum groups start their kc0 matmuls as soon as s[kc0]
        # is ready (kc1 accumulates later), filling the PE while the kc1
        # S-chain finishes.
        early = []  # (ps, t, g) with kc0 already accumulated
        for t, g in ((0, 0), (0, 1), (1, 0)):
            ps = psum.tile([128, 1024], _F32, tag="ps")
            for nn in range(2):
                c0 = g * 1024 + nn * 512
                nc.tensor.matmul(
                    ps[:, nn * 512 : (nn + 1) * 512],
                    lhsT=s_sb[:, 0, t * 128 : (t + 1) * 128],
                    rhs=TT[:, 0, c0 : c0 + 512],
                    start=True,
                    stop=False,
                )
            early.append(((t, g), ps))
        early = dict(early)

        # ---- tail groups (cols 8192..8319) for all tasks: done early --------
        tstage = tstage_pool.tile([128, NTASK, 128], _BF16)
        for t in range(NTASK):
            ps = upsum.tile([128, 128], _F32, tag="ups")
            nc.tensor.matmul(
                ps,
                lhsT=s_sb[:, 0, t * 128 : (t + 1) * 128],
                rhs=TT[:, 0, TAILCOL:],
                start=True,
                stop=False,
            )
            nc.tensor.matmul(
                ps,
                lhsT=s_sb[:, 1, t * 128 : (t + 1) * 128],
                rhs=TT[:, 1, TAILCOL:],
                start=False,
                stop=True,
            )
            nc.scalar.copy(tstage[:, t, :], ps)
        nc.sync.dma_start(out[:, :, TAILCOL:].rearrange("t b c -> b t c"), tstage)

        # ---- main loop: 4 gp x 3 tasks x 2 groups; prefetch next TT chunks --
        cp_i = 0
        for gp in range(4):
            if gp + 1 < NCHUNK:
                for par in range(2):
                    for kc in range(2):
                        tt_chunk(gp + 1, par, kc)
            for t in range(NTASK):
                last = gp == 3 and t == NTASK - 1
                stg = stage_pool.tile([128, 2048], _BF16)
                for half in range(2):
                    g = gp * 2 + half
                    col0 = g * 1024
                    pre = early.get((t, g))
                    ps = pre if pre is not None else psum.tile(
                        [128, 1024], _F32, tag="ps"
                    )
                    for nn in range(2):
                        sl = slice(nn * 512, (nn + 1) * 512)
                        c0 = col0 + nn * 512
                        if pre is None:
                            nc.tensor.matmul(
                                ps[:, sl],
                                lhsT=s_sb[:, 0, t * 128 : (t + 1) * 128],
                                rhs=TT[:, 0, c0 : c0 + 512],
                                start=True,
                                stop=False,
                            )
                        nc.tensor.matmul(
                            ps[:, sl],
                            lhsT=s_sb[:, 1, t * 128 : (t + 1) * 128],
                            rhs=TT[:, 1, c0 : c0 + 512],
                            start=False,
                            stop=True,
                        )
                    dst = stg[:, half * 1024 : (half + 1) * 1024]
                    if last:
                        # split the final copies 512-wide across both engines
                        # so the drain runs in parallel
                        nc.scalar.copy(dst[:, 0:512], ps[:, 0:512])
                        nc.vector.tensor_copy(
                            out=dst[:, 512:1024], in_=ps[:, 512:1024]
                        )
                        nc.sync.dma_start(
                            out[t, :, gp * 2048 + half * 1024 :
                                gp * 2048 + (half + 1) * 1024],
                            dst,
                        )
                    else:
                        if CP_SCHED[cp_i] == "A":
                            nc.scalar.copy(dst, ps)
                        else:
                            nc.vector.tensor_copy(out=dst, in_=ps)
                    cp_i += 1
                if not last:
                    nc.sync.dma_start(out[t, :, gp * 2048 : (gp + 1) * 2048], stg)


_NC_CACHE = {}


def _core_tasks(c):
    i = c // 2
    js = [j for j in range(N) if j != i]
    halves = [(j, h) for j in js for h in (0, 1)]
    return i, (halves[0:3] if c % 2 == 0 else halves[3:6])


def _build():
    key = "v4"
    if key in _NC_CACHE:
        return _NC_CACHE[key]
    nc = bacc.Bacc("TRN2", target_bir_lowering=False, debug=False, num_devices=NCORES)
    w1x = nc.dram_tensor("w1x", [128, 2, 2, 256], _BF16, kind="ExternalInput").ap()
    w1zt = nc.dram_tensor("w1zt", [128, 640], _BF16, kind="ExternalInput").ap()
    cvec = nc.dram_tensor("cvec", [128, 6], _F32, kind="ExternalInput").ap()
    out = nc.dram_tensor("out", [NTASK, HALF, COLS], _BF16, kind="ExternalOutput").ap()
    with tile.TileContext(nc) as tc:
        _emit(tc, nc, w1x, w1zt, cvec, out)
    nc.compile()
    _NC_CACHE[key] = nc
    return nc


def _slot_col(d):
    """Column of diagonal-slot d in the packed layout."""
    if d == 64:
        return TAILCOL
    if d % 2 == 0:
        de = d // 2
        return (de // 8) * 2048 + (de % 8) * 128
    do = (d - 1) // 2
    return (do // 8) * 2048 + 1024 + (do % 8) * 128


_LUT = None


def _lut():
    global _LUT
    if _LUT is None:
        a = np.arange(128)[:, None]
        c = np.arange(128)[None, :]
        g = (c - a) % 128
        d = np.where(g <= 64, g, 128 - g)
        base_a = np.where(g <= 64, np.broadcast_to(a, (128, 128)), c)
        slot = np.vectorize(_slot_col)(d)
        _LUT = (slot + base_a).astype(np.int32)
    return _LUT


# Options for test harness introspection (set by test.py, unused in grading).
_RUN_KWARGS = {}
_LAST_RESULT = None


def kernel(z_all, W1, b1, W2, b2):
    global _LAST_RESULT
    z_all = np.asarray(z_all, dtype=np.float32)
    W1 = np.asarray(W1, dtype=np.float32)
    b1 = np.asarray(b1, dtype=np.float32)
    W2 = np.asarray(W2, dtype=np.float32)

    nc = _build()
    bf = ml_dtypes.bfloat16

    in_maps = []
    metas = []
    for c in range(NCORES):
        i, tasks = _core_tasks(c)
        metas.append((i, tasks))
        w1i = W1[i].astype(bf)  # [256, 128]
        w1ck = w1i.reshape(2, 128, 128).transpose(1, 0, 2)  # [k%128, kc, a]
        w1dbl = np.concatenate([w1ck, w1ck], axis=2)  # [128, 2, 256]
        w1shf = np.concatenate(
            [w1ck[:, :, 1:], w1ck[:, :, :1], w1ck[:, :, 1:], w1ck[:, :, :1]], axis=2
        )  # shifted: w1o[p, kc, j] = w1[p, kc, (j+1)%128]
        w1x = np.stack([w1dbl, w1shf], axis=1)  # [128, 2, 2, 256]
        ztm = np.concatenate(
            [z_all[j, h * HALF : (h + 1) * HALF, :].T for (j, h) in tasks], axis=1
        ).astype(bf)  # [128 d, 384]
        w1zt = np.concatenate([w1i.T, ztm], axis=1)  # [128, 640]
        cvec = np.stack(
            [
                b1[i][:128], b1[i][128:],
                -2.0 * W2[i, 0, :128], -2.0 * W2[i, 0, 128:],
                2.0 * W2[i, 0, :128], 2.0 * W2[i, 0, 128:],
            ],
            axis=1,
        ).astype(np.float32)  # [128, 6]
        in_maps.append(
            {
                "w1x": np.ascontiguousarray(w1x),
                "w1zt": np.ascontiguousarray(w1zt),
                "cvec": np.ascontiguousarray(cvec),
            }
        )

    res = run_bass_kernel_spmd(nc, in_maps, list(range(NCORES)), **_RUN_KWARGS)
    _LAST_RESULT = res

    lut = _lut()
    full = np.zeros((N, N, B, D, D), dtype=np.float32)
    for c in range(NCORES):
        i, tasks = metas[c]
        o = np.asarray(res.results[c]["out"]).astype(np.float32)  # [NTASK, HALF, COLS]
        for t, (j, h) in enumerate(tasks):
            full[i, j, h * HALF : (h + 1) * HALF] = o[t][:, lut]
    return full


# revision 28
# speedup vs baseline: 1.2080x; 1.0356x over previous
"""Trainium2 Bass kernel for nn_GameTensor_27195732918735.

Computes out[i,j,b] = Hessian_z V_i(z_all[j,b]) for i != j, zeros on the
diagonal, where V_i(z) = W2[i] @ tanh(W1[i] @ z + b1[i]) + b2[i].

Analytic form used on-device:
    u = W1 z + b1;  th = tanh(u);  s_k = -2 W2_k th_k (1 - th_k^2)
    H = W1^T diag(s) W1  =  sum_k s_k w1_k w1_k^T

H is symmetric, so the device only computes one entry per unordered pair
(a, c).  Pairs are packed by circular diagonal: slot d in 0..64 holds
T[k, d, a] = W1[k, a] * W1[k, (a + d) % 128], built on DVE from a doubled
copy of W1 (plus a one-shifted copy for odd d, keeping every operand
stride-1 and 4B-aligned so the bf16 2x_1P perf mode engages).  Per task the
Hessians for 128 batches are then H[b, col] = sum_k S[k, b] T[k, col]
(bf16 matmuls, fp32 PSUM), staged to SBUF as bf16 and DMAd out.  The host
mirrors the packed pairs into the full [B, D, D] blocks with a gather LUT
and writes the diagonal zero blocks (both pure data movement).

Engine notes baked into the structure (measured on TRN2):
  - GPSIMD is unused: it cannot read PSUM, and any concurrent GPSIMD
    execution slows DVE/Scalar ops several-fold.
  - Only Scalar and Vector can read PSUM; the 24 group copies are split
    between them (CP_SCHED), Scalar-heavy early while DVE builds TT.
  - Input DMAs are merged into 3 transfers to cut Sync issue latency.
  - PE warmup matmuls ramp the p-state before real work arrives; a dummy
    tanh preloads the activation table.

Per-core plan (8 cores, SPMD): core c owns agent i = c//2 and three
(j, batch-half) tasks (12 nonzero (i,j) cells x 2 halves = 24 / 8 = 3).
"""

import numpy as np
import ml_dtypes

import concourse.bass as bass
import concourse.mybir as mybir
import concourse.tile as tile
from concourse import bacc
from concourse.bass_utils import run_bass_kernel_spmd

# ---- custom fused DVE op: s = C0 * (x - x^3) = C0 * x * (1 - x^2) ----------
import concourse.dve_ops as _dve_ops
from concourse.dve_ops import DveOp as _DveOp, DveOpSpec as _DveOpSpec, OPS as _OPS
from concourse.dve_spec import Spec as _Spec, Src0 as _Src0, C0 as _C0
from concourse.dve_spec import lower as _dve_lower


def _register_tanh_hess_op():
    name = "TANH_HESS_S_ANT"
    for op in _OPS:
        if op.name == name:
            return op
    spec = _Spec(
        body=(_Src0 - _Src0 * _Src0 * _Src0) * _C0,
        reference=lambda in0, s0: (in0 - in0**3) * s0,
    )
    _dve_ops._SUB_OPCODE_FOR_NAME[name] = _dve_ops._CUSTOM_DVE_ROW_BASE + len(_OPS)
    shas = {}
    for ver in ("v3", "v4"):
        s = _DveOpSpec(
            name=name,
            opcode=_dve_ops._SUB_OPCODE_FOR_NAME[name],
            uops=_dve_lower(spec, ver=ver),
            rd1_en=False,
        )
        shas[ver] = s.sha(ver)
    op = _DveOp(name, spec, subdim=False, uops_sha=shas)
    _OPS.append(op)
    _dve_ops.CUSTOM_DVE_SPECS[name] = spec
    return op


_TANH_HESS_S = _register_tanh_hess_op()

N, B, D = 4, 256, 128
H2 = 2 * D  # 256 hidden
NCORES = 8
NTASK = 3  # (j, half) tasks per core
HALF = B // 2  # 128 batches per task

# Packed-pair layout: 65 diagonal slots of 128 columns.
# Column order: [E0 O0 E1 O1 E2 O2 E3 O3 | TAIL] where E-chunk e holds even
# d = 16e..16e+14 (8 slots), O-chunk o holds odd d = 16o+1..16o+15 (8 slots),
# TAIL is the single d=64 slot. Total 8*1024 + 128 = 8320 columns.
NSLOT = 65
COLS = NSLOT * 128  # 8320
NCHUNK = 4  # E/O chunk pairs
TAILCOL = 8192

MM_MODE = "bf16"  # kept for test-harness compat; bf16 is the only mode

_F32 = mybir.dt.float32
_BF16 = mybir.dt.bfloat16

_AP = None  # bass_rust.AP class, resolved lazily


def _win_ap(tile_ap, base_off, nd, dstep):
    """Overlapping sliding-window AP: [128p][nd windows, stride dstep][128, 1].

    tile_ap must be a [128, R] view of an SBUF tile. Window w reads elements
    base_off + w*dstep + 0..127 of the view.
    """
    global _AP
    if _AP is None:
        _AP = type(tile_ap)
    pdim = [int(v) for v in list(tile_ap.ap)[0]]
    return _AP(
        tensor=tile_ap.tensor,
        offset=int(tile_ap.offset) + base_off,
        ap=[pdim, [dstep, nd], [1, 128]],
    )


# 24 big PSUM->SBUF copies: A = Scalar (closer to PSUM), D = Vector.
# Scalar-heavy early (DVE still building TT chunks), balanced later; the
# final stage's pair is A,D so the two copies run in parallel at the end.
CP_SCHED = "AADAAD" "AADAAD" "ADADAD" "ADADAD"


def _emit(tc, nc, w1x, w1zt, cvec, out):
    Tanh = mybir.ActivationFunctionType.Tanh
    Ident = mybir.ActivationFunctionType.Identity
    mult = mybir.AluOpType.mult
    add = mybir.AluOpType.add

    with (
        tc.tile_pool(name="consts", bufs=1) as consts,
        tc.tile_pool(name="tpool", bufs=1) as tpool,
        tc.tile_pool(name="small", bufs=2) as small,
        tc.tile_pool(name="warm", bufs=1) as warm,
        tc.tile_pool(name="stage", bufs=6) as stage_pool,
        tc.tile_pool(name="tstage", bufs=1) as tstage_pool,
        tc.tile_pool(name="upsum", bufs=2, space="PSUM") as upsum,
        tc.tile_pool(name="psum", bufs=3, space="PSUM") as psum,
    ):
        # ---- merged input DMAs ----------------------------------------------
        w1zt_sb = consts.tile([128, 640], _BF16)  # [d, w1t(256) | zt(3x128)]
        nc.scalar.dma_start(w1zt_sb, w1zt)
        cv_sb = consts.tile([128, 6], _F32)  # [b1(2) | w2s(2) | w2n(2)]
        nc.scalar.dma_start(cv_sb, cvec)
        w1x_sb = consts.tile([128, 2, 2, 256], _BF16)  # [p, dbl/shf, kc, a]
        nc.sync.dma_start(w1x_sb, w1x)

        # ---- warmups: PE p-state ramp + Tanh act-table preload --------------
        wz = warm.tile([128, 512], _BF16)
        nc.vector.memset(wz, 0)
        for _ in range(5):
            wps = psum.tile([128, 1024], _F32, tag="ps")
            nc.tensor.matmul(
                wps[:, 0:512], lhsT=wz[:, 0:128], rhs=wz, start=True, stop=True
            )
        wt = warm.tile([128, 8], _F32)
        nc.scalar.memzero(wt)
        wth = warm.tile([128, 8], _F32)
        nc.scalar.activation(wth, wt, Tanh, bias=0.0)

        TT = tpool.tile([128, 2, COLS], _BF16)

        def tt_chunk(ch, par, kc):
            col0 = ch * 2048 + par * 1024
            dst = TT[:, kc, col0 : col0 + 1024].rearrange("p (w x) -> p w x", w=8)
            in0 = w1x_sb[:, 0, kc, None, 0:128].to_broadcast((128, 8, 128))
            in1 = _win_ap(w1x_sb[:, par, kc, :], 16 * ch, 8, 2)
            nc.vector.tensor_tensor(dst, in0, in1, mult)

        # ---- TT: tail slot (d=64) + first chunk pair on DVE -----------------
        for kc in range(2):
            nc.vector.tensor_tensor(
                TT[:, kc, TAILCOL : TAILCOL + 128],
                w1x_sb[:, 0, kc, 0:128],
                w1x_sb[:, 0, kc, 64:192],
                mult,
            )
        for kc in range(2):
            tt_chunk(0, 0, kc)  # E0
        for kc in range(2):
            tt_chunk(0, 1, kc)  # O0

        # ---- S batched over tasks: s = th * (w2s*(1 - th^2)) ----------------
        # th, sq = th^2, g = w2s - w2s*sq on Scalar (per-partition scale and
        # bias APs); one DVE multiply s = th*g per kc.
        s_sb = consts.tile([128, 2, NTASK * 128], _BF16)
        for kc in range(2):
            ups = upsum.tile([128, NTASK * 128], _F32, tag="ups")
            nc.tensor.matmul(
                ups,
                lhsT=w1zt_sb[:, kc * 128 : (kc + 1) * 128],
                rhs=w1zt_sb[:, 256:640],
                start=True,
                stop=True,
            )
            th = small.tile([128, NTASK * 128], _F32, tag="th")
            nc.scalar.activation(th, ups, Tanh, bias=cv_sb[:, kc : kc + 1])
            with tc.high_priority():
                # fused: s = w2s * (th - th^3) in one DVE pass
                nc.vector._custom_dve(
                    _TANH_HESS_S,
                    out=s_sb[:, kc, :],
                    in0=th,
                    s0=cv_sb[:, 2 + kc : 3 + kc],
                )

        # extra warmups keep the PE p-state hot until real groups arrive
        for _ in range(8):
            wps = psum.tile([128, 1024], _F32, tag="ps")
            nc.tensor.matmul(
                wps[:, 0:512], lhsT=wz[:, 0:128], rhs=wz, start=True, stop=True
            )

        # The first 3 psum groups start their kc0 matmuls as soon as s[kc0]
        # is ready (kc1 accumulates later), filling the PE while the kc1
        # S-chain finishes.
        early = []  # (ps, t, g) with kc0 already accumulated
        for t, g in ((0, 0), (0, 1), (1, 0)):
            ps = psum.tile([128, 1024], _F32, tag="ps")
            for nn in range(2):
                c0 = g * 1024 + nn * 512
                nc.tensor.matmul(
                    ps[:, nn * 512 : (nn + 1) * 512],
                    lhsT=s_sb[:, 0, t * 128 : (t + 1) * 128],
                    rhs=TT[:, 0, c0 : c0 + 512],
                    start=True,
                    stop=False,
                )
            early.append(((t, g), ps))
        early = dict(early)

        # ---- tail groups (cols 8192..8319) for all tasks: done early --------
        tstage = tstage_pool.tile([128, NTASK, 128], _BF16)
        for t in range(NTASK):
            ps = upsum.tile([128, 128], _F32, tag="ups")
            nc.tensor.matmul(
                ps,
                lhsT=s_sb[:, 0, t * 128 : (t + 1) * 128],
                rhs=TT[:, 0, TAILCOL:],
                start=True,
                stop=False,
            )
            nc.tensor.matmul(
                ps,
                lhsT=s_sb[:, 1, t * 128 : (t + 1) * 128],
                rhs=TT[:, 1, TAILCOL:],
                start=False,
                stop=True,
            )
            nc.scalar.copy(tstage[:, t, :], ps)
        nc.sync.dma_start(out[:, :, TAILCOL:].rearrange("t b c -> b t c"), tstage)

        # ---- main loop: 4 gp x 3 tasks x 2 groups; prefetch next TT chunks --
        cp_i = 0
        for gp in range(4):
            if gp + 1 < NCHUNK:
                for par in range(2):
                    for kc in range(2):
                        tt_chunk(gp + 1, par, kc)
            for t in range(NTASK):
                last = gp == 3 and t == NTASK - 1
                stg = stage_pool.tile([128, 2048], _BF16)
                for half in range(2):
                    g = gp * 2 + half
                    col0 = g * 1024
                    pre = early.get((t, g))
                    ps = pre if pre is not None else psum.tile(
                        [128, 1024], _F32, tag="ps"
                    )
                    for nn in range(2):
                        sl = slice(nn * 512, (nn + 1) * 512)
                        c0 = col0 + nn * 512
                        if pre is None:
                            nc.tensor.matmul(
                                ps[:, sl],
                                lhsT=s_sb[:, 0, t * 128 : (t + 1) * 128],
                                rhs=TT[:, 0, c0 : c0 + 512],
                                start=True,
                                stop=False,
                            )
                        nc.tensor.matmul(
                            ps[:, sl],
                            lhsT=s_sb[:, 1, t * 128 : (t + 1) * 128],
                            rhs=TT[:, 1, c0 : c0 + 512],
                            start=False,
                            stop=True,
                        )
                    dst = stg[:, half * 1024 : (half + 1) * 1024]
                    if last:
                        # split the final copies 512-wide across both engines
                        # so the drain runs in parallel
                        nc.scalar.copy(dst[:, 0:512], ps[:, 0:512])
                        nc.vector.tensor_copy(
                            out=dst[:, 512:1024], in_=ps[:, 512:1024]
                        )
                        nc.sync.dma_start(
                            out[t, :, gp * 2048 + half * 1024 :
                                gp * 2048 + (half + 1) * 1024],
                            dst,
                        )
                    else:
                        if CP_SCHED[cp_i] == "A":
                            nc.scalar.copy(dst, ps)
                        else:
                            nc.vector.tensor_copy(out=dst, in_=ps)
                    cp_i += 1
                if not last:
                    nc.sync.dma_start(out[t, :, gp * 2048 : (gp + 1) * 2048], stg)


_NC_CACHE = {}


def _core_tasks(c):
    i = c // 2
    js = [j for j in range(N) if j != i]
    halves = [(j, h) for j in js for h in (0, 1)]
    return i, (halves[0:3] if c % 2 == 0 else halves[3:6])


def _build():
    key = "v4"
    if key in _NC_CACHE:
        return _NC_CACHE[key]
    nc = bacc.Bacc("TRN2", target_bir_lowering=False, debug=False, num_devices=NCORES)
    w1x = nc.dram_tensor("w1x", [128, 2, 2, 256], _BF16, kind="ExternalInput").ap()
    w1zt = nc.dram_tensor("w1zt", [128, 640], _BF16, kind="ExternalInput").ap()
    cvec = nc.dram_tensor("cvec", [128, 6], _F32, kind="ExternalInput").ap()
    out = nc.dram_tensor("out", [NTASK, HALF, COLS], _BF16, kind="ExternalOutput").ap()
    with tile.TileContext(nc) as tc:
        _emit(tc, nc, w1x, w1zt, cvec, out)
    nc.compile()
    _NC_CACHE[key] = nc
    return nc


def _slot_col(d):
    """Column of diagonal-slot d in the packed layout."""
    if d == 64:
        return TAILCOL
    if d % 2 == 0:
        de = d // 2
        return (de // 8) * 2048 + (de % 8) * 128
    do = (d - 1) // 2
    return (do // 8) * 2048 + 1024 + (do % 8) * 128


_LUT = None


def _lut():
    global _LUT
    if _LUT is None:
        a = np.arange(128)[:, None]
        c = np.arange(128)[None, :]
        g = (c - a) % 128
        d = np.where(g <= 64, g, 128 - g)
        base_a = np.where(g <= 64, np.broadcast_to(a, (128, 128)), c)
        slot = np.vectorize(_slot_col)(d)
        _LUT = (slot + base_a).astype(np.int32)
    return _LUT


# Options for test harness introspection (set by test.py, unused in grading).
_RUN_KWARGS = {}
_LAST_RESULT = None


def kernel(z_all, W1, b1, W2, b2):
    global _LAST_RESULT
    z_all = np.asarray(z_all, dtype=np.float32)
    W1 = np.asarray(W1, dtype=np.float32)
    b1 = np.asarray(b1, dtype=np.float32)
    W2 = np.asarray(W2, dtype=np.float32)

    nc = _build()
    bf = ml_dtypes.bfloat16

    in_maps = []
    metas = []
    for c in range(NCORES):
        i, tasks = _core_tasks(c)
        metas.append((i, tasks))
        w1i = W1[i].astype(bf)  # [256, 128]
        w1ck = w1i.reshape(2, 128, 128).transpose(1, 0, 2)  # [k%128, kc, a]
        w1dbl = np.concatenate([w1ck, w1ck], axis=2)  # [128, 2, 256]
        w1shf = np.concatenate(
            [w1ck[:, :, 1:], w1ck[:, :, :1], w1ck[:, :, 1:], w1ck[:, :, :1]], axis=2
        )  # shifted: w1o[p, kc, j] = w1[p, kc, (j+1)%128]
        w1x = np.stack([w1dbl, w1shf], axis=1)  # [128, 2, 2, 256]
        ztm = np.concatenate(
            [z_all[j, h * HALF : (h + 1) * HALF, :].T for (j, h) in tasks], axis=1
        ).astype(bf)  # [128 d, 384]
        w1zt = np.concatenate([w1i.T, ztm], axis=1)  # [128, 640]
        cvec = np.stack(
            [
                b1[i][:128], b1[i][128:],
                -2.0 * W2[i, 0, :128], -2.0 * W2[i, 0, 128:],
                2.0 * W2[i, 0, :128], 2.0 * W2[i, 0, 128:],
            ],
            axis=1,
        ).astype(np.float32)  # [128, 6]
        in_maps.append(
            {
                "w1x": np.ascontiguousarray(w1x),
                "w1zt": np.ascontiguousarray(w1zt),
                "cvec": np.ascontiguousarray(cvec),
            }
        )

    res = run_bass_kernel_spmd(nc, in_maps, list(range(NCORES)), **_RUN_KWARGS)
    _LAST_RESULT = res

    lut = _lut()
    full = np.zeros((N, N, B, D, D), dtype=np.float32)
    for c in range(NCORES):
        i, tasks = metas[c]
        o = np.asarray(res.results[c]["out"]).astype(np.float32)  # [NTASK, HALF, COLS]
        for t, (j, h) in enumerate(tasks):
            full[i, j, h * HALF : (h + 1) * HALF] = o[t][:, lut]
    return full
